# revision 14
# baseline (speedup 1.0000x reference)
"""Trainium2 Bass kernel for a 3-layer complex RBF network.

Math per layer (complex y, G; real phi):
    dist_i = sum_j |y_j - G_ij|^2
    phi    = exp(-dist / (2 s))
    y_out  = W @ phi + b        (complex W, b)

Distribution (8 cores): shard the hidden axis I=4096 -> 512 rows of G / columns
of W per core.  dist/phi are computed fully locally per shard; the matvec
W[:, shard] @ phi_shard yields a full-length partial y that is AllReduce-summed
across cores (b is added as b/8 on every core's partial before the reduce).

Per-core implementation (NeuronCore engines):
  - G rows arrive natural-layout [128 i x Oprev] via SWDGE cast-DMA (fp32->bf16).
  - y is replicated across partitions (DMA broadcast) once per layer, then
    DVE computes diff = G - y_bcast in place; ACT computes Square(diff) with
    accum_out, giving dist partials [128,1] per chunk -- no G-side transposes.
  - phi = ACT Exp(clamp(dist * (-1/(2s)))) lands as [128,1] chunks: exactly the
    stationary (lhsT) layout the PE matvec needs.
  - W tiles [128 o x 512 i] are cast-DMA'd to bf16 and xbar-transposed
    (HWDGE, 128x128 blocks) into [128 i x 128 o] rhs tiles; PE accumulates
    y_partial[1, o] over the 4 i-chunks in PSUM.
  - AllReduce (gpsimd/ncfw) sums partial y across the 8 cores.

Weights are read from HBM exactly once (fp32) -- the memory roofline.

Engine-ring ordering notes: weight loads live on the gpsimd (SWDGE) ring and
are emitted so that the next layer's stream sits *before* the current layer's
AllReduce, keeping prefetch flowing; y plumbing (ccin store, y reload,
broadcast) lives on the scalar HWDGE ring; xbar transposes own the sync ring.
"""

import numpy as np

P = 128
NCORES = 8
HID = 4096
IS = HID // NCORES          # 512: per-core shard of the hidden axis
NCH = IS // P               # 4 chunks of 128
# (Oprev, Ol) for layers 1..3
DIMS = [(1024, 4096), (4096, 4096), (4096, 1024)]

_cache = {}


def _build_nc():
    import concourse.bacc as bacc
    import concourse.mybir as mybir
    import concourse.tile as tile

    f32 = mybir.dt.float32
    bf16 = mybir.dt.bfloat16
    AF = mybir.ActivationFunctionType
    ALU = mybir.AluOpType

    # Bacc (not raw Bass): its compile() pass legalizes multi-sem waits into
    # InstEventSemaphore carriers (HW instructions hold only 1 wait slot).
    nc = bacc.Bacc(None)

    x = nc.dram_tensor("x", [2, 1024], f32, kind="ExternalInput")
    W, G, S, B = {}, {}, {}, {}
    for l, (Op, Ol) in enumerate(DIMS, start=1):
        W[l] = nc.dram_tensor(f"W{l}s", [2, Ol, IS], f32, kind="ExternalInput")
        G[l] = nc.dram_tensor(f"G{l}s", [2, IS, Op], f32, kind="ExternalInput")
        S[l] = nc.dram_tensor(f"s{l}s", [IS], f32, kind="ExternalInput")
        B[l] = nc.dram_tensor(f"b{l}f", [2, Ol], f32, kind="ExternalInput")
    out = nc.dram_tensor("out", [2, 1024], f32, kind="ExternalOutput")

    with tile.TileContext(nc) as tc:
        with (
            tc.tile_pool(name="gnat", bufs=5) as gnat,      # [128, Op] bf16 natural G
            tc.tile_pool(name="wnat", bufs=8) as wnatp,     # [128, 512] bf16 natural W
            tc.tile_pool(name="wT", bufs=192) as wTp,       # [128, 128] bf16 transposed W
            tc.tile_pool(name="ybc", bufs=3) as ybcp,       # [128, Op] bf16 replicated y
            tc.tile_pool(name="small", bufs=1) as small,
            tc.tile_pool(name="psum", bufs=4, space="PSUM") as psump,
            tc.tile_pool(name="dram", bufs=1, space="DRAM") as dramp,
        ):
            # ---------------- preloads ----------------------------------------
            # s -> -1/(2s) as [128, NCH] chunks; b -> b/8 staged in DRAM
            n2s, b8d = {}, {}
            for l, (Op, Ol) in enumerate(DIMS, start=1):
                s4 = small.tile([P, NCH], f32, tag=f"s4_{l}")
                nc.gpsimd.dma_start(s4[:], S[l][:].rearrange("(c p) -> p c", p=P))
                rec = small.tile([P, NCH], f32, tag=f"rec_{l}")
                nc.vector.reciprocal(rec[:], s4[:])
                t = small.tile([P, NCH], f32, tag=f"n2s_{l}")
                nc.vector.tensor_scalar_mul(t[:], rec[:], -0.5)
                n2s[l] = t

                # b/8 staged in DRAM (single-row scratch: partition-1 DVE
                # accesses are rejected by the BIR verifier)
                bsc = small.tile([1, 2 * Ol], f32, tag="ysb")  # reuse ysb scratch
                nc.scalar.dma_start(bsc[:], B[l][:])
                nc.vector.tensor_scalar_mul(bsc[:], bsc[:], 1.0 / NCORES)
                bd = dramp.tile([2, Ol], f32, tag=f"b8_{l}")
                nc.scalar.dma_start(bd[:], bsc[:])
                b8d[l] = bd

            # scratch used by "touch" ops that pre-absorb DMA-completion waits
            # onto the DVE engine clock (ISA instructions have very few
            # sync-wait slots; a converging 3-sem fan-in fails codegen)
            touch = small.tile([1, 2], bf16, tag="touch")

            # ---------------- y broadcast for layer 1 (from x, cast) ----------
            ybct = {}
            for r in range(2):
                yb = ybcp.tile([P, DIMS[0][0]], bf16, tag="ybc")
                nc.gpsimd.dma_start(yb[:], x[r : r + 1, :].partition_broadcast(P))
                nc.vector.tensor_copy(touch[:, r : r + 1], yb[0:1, 0:1])
                ybct[(1, r)] = yb

            # ---------------- weight-load emission helpers --------------------
            gt = {}    # (l, c, r) -> natural G tile
            wTt = {}   # (l, r, ot, c) -> transposed W tile [128 i, 128 o]

            def emit_g_loads(l):
                Op = DIMS[l - 1][0]
                for c in range(NCH):
                    for r in range(2):
                        g = gnat.tile([P, Op], bf16, tag="gnat")
                        nc.gpsimd.dma_start(g[:], G[l][r, c * P : (c + 1) * P, :])
                        gt[(l, c, r)] = g

            def emit_w_loads(l, ots):
                """SWDGE cast-load W natural tiles and xbar-transpose them."""
                for r, ot in ots:
                    wn = wnatp.tile([P, IS], bf16, tag="wnat")
                    nc.gpsimd.dma_start(wn[:], W[l][r, ot * P : (ot + 1) * P, :])
                    for c in range(NCH):
                        wt = wTp.tile([P, P], bf16, tag="wT")
                        nc.sync.dma_start(
                            wt[:], wn[:, c * P : (c + 1) * P], transpose=True
                        )
                        wTt[(l, r, ot, c)] = wt

            def w_ots(l, lo, hi):
                Ol = DIMS[l - 1][1]
                all_ots = [(r, ot) for r in range(2) for ot in range(Ol // P)]
                return all_ots[lo:hi]

            # Prefetch order on the gpsimd ring:
            # G1, W1, G2, W2[:12] | b1acc AR1 | W2[12:], G3, W3[:12] | b2acc AR2 | W3[12:] | b3acc AR3
            emit_g_loads(1)
            emit_w_loads(1, w_ots(1, 0, 10**9))
            emit_g_loads(2)
            emit_w_loads(2, w_ots(2, 0, 12))

            # ---------------- per-layer compute --------------------------------
            ysrc = None  # DRAM bf16 [2, Op] holding y of the previous layer
            for l, (Op, Ol) in enumerate(DIMS, start=1):
                NOS = Ol // 512

                if l > 1:
                    for r in range(2):
                        yb = ybcp.tile([P, Op], bf16, tag="ybc")
                        nc.scalar.dma_start(
                            yb[:], ysrc[r : r + 1, :].partition_broadcast(P)
                        )
                        nc.vector.tensor_copy(touch[:, r : r + 1], yb[0:1, 0:1])
                        ybct[(l, r)] = yb

                # ---- dist: g = G - y_bcast (DVE, in place), Square+accum (ACT)
                dacc = small.tile([P, 2 * NCH], f32, tag=f"dacc_{l}")
                for c in range(NCH):
                    for r in range(2):
                        g = gt[(l, c, r)]
                        nc.vector.tensor_sub(g[:], g[:], ybct[(l, r)][:])
                        nc.scalar.activation(
                            g[:], g[:], AF.Square,
                            accum_out=dacc[:, 2 * c + r : 2 * c + r + 1],
                        )

                # ---- phi = exp(clamp((d_re+d_im) * -1/(2s), -85)) ----
                phi = small.tile([P, NCH], bf16, tag=f"phi_{l}")
                expin = small.tile([P, NCH], f32, tag=f"expin_{l}")
                junk2 = small.tile([P, 2], f32, tag=f"junk_{l}")
                for c in range(NCH):
                    nc.vector.tensor_scalar(
                        junk2[:], dacc[:, 2 * c : 2 * c + 2],
                        n2s[l][:, c : c + 1], 0.0, ALU.mult, ALU.add,
                        accum_out=expin[:, c : c + 1],
                    )
                    nc.vector.tensor_scalar_max(
                        expin[:, c : c + 1], expin[:, c : c + 1], -85.0
                    )
                    nc.scalar.activation(
                        phi[:, c : c + 1], expin[:, c : c + 1], AF.Exp
                    )

                # ---- y_partial = W_shard @ phi (PE), PSUM -> SBUF (DVE) ----
                # single-row layout [1, 2*Ol]: row = comp0 ++ comp1
                ysb = small.tile([1, 2 * Ol], f32, tag="ysb")
                for r in range(2):
                    for os_ in range(NOS):
                        ps = psump.tile([1, 512], f32, tag="psy")
                        for sub in range(4):
                            ot = os_ * 4 + sub
                            for c in range(NCH):
                                nc.tensor.matmul(
                                    ps[:, sub * P : (sub + 1) * P],
                                    phi[:, c : c + 1],
                                    wTt[(l, r, ot, c)][:],
                                    start=(c == 0),
                                    stop=(c == NCH - 1),
                                )
                        off = r * Ol + os_ * 512
                        # scalar engine: merges the matmuls' phi-wait and the
                        # psum-slot WAR wait onto one engine clock (wait-slot
                        # budget), and ScE reads PSUM faster than DVE
                        nc.scalar.copy(ysb[:, off : off + 512], ps[:])

                # ---- partial y + b/8, then AllReduce across the 8 cores ----
                ccin = dramp.tile([2, Ol], f32, tag=f"ccin_{l}")
                ccout = dramp.tile([2, Ol], f32, tag=f"ccout_{l}")
                nc.scalar.dma_start(ccin[:], ysb[:])
                nc.gpsimd.dma_start(ccin[:], b8d[l][:], accum_op=ALU.add)
                nc.gpsimd.collective_compute(
                    "AllReduce",
                    ALU.add,
                    replica_groups=[list(range(NCORES))],
                    ins=[ccin.opt()],
                    outs=[ccout.opt()],
                )
                # keep next-layer weight prefetch flowing on the gpsimd ring
                if l == 1:
                    emit_w_loads(2, w_ots(2, 12, 10**9))
                    emit_g_loads(3)
                    emit_w_loads(3, w_ots(3, 0, 12))
                elif l == 2:
                    emit_w_loads(3, w_ots(3, 12, 10**9))

                if l < 3:
                    yt = small.tile([1, 2 * Ol], f32, tag="yt")
                    nc.scalar.dma_start(yt[:], ccout[:])
                    ybfsb = small.tile([1, 2 * Ol], bf16, tag="ybfsb")
                    nc.vector.tensor_copy(ybfsb[:], yt[:])
                    ybf = dramp.tile([2, Ol], bf16, tag=f"ybf_{l}")
                    nc.scalar.dma_start(ybf[:], ybfsb[:])
                    ysrc = ybf
                else:
                    nc.gpsimd.dma_start(out[:], ccout[:])

    # Bacc.finalize runs compile(): reg alloc + event-semaphore legalization
    nc.finalize()
    return nc


def _get_nc():
    if "nc" not in _cache:
        _cache["nc"] = _build_nc()
    return _cache["nc"]


def make_in_maps(inputs):
    """Host-side sharding: slice the hidden axis into 8 shards."""
    in_maps = []
    for c in range(NCORES):
        lo, hi = c * IS, (c + 1) * IS
        m = {"x": np.ascontiguousarray(inputs["x"], dtype=np.float32)}
        for l in range(1, 4):
            m[f"W{l}s"] = np.ascontiguousarray(inputs[f"W{l}"][:, :, lo:hi], dtype=np.float32)
            m[f"G{l}s"] = np.ascontiguousarray(inputs[f"G{l}"][:, lo:hi, :], dtype=np.float32)
            m[f"s{l}s"] = np.ascontiguousarray(inputs[f"s{l}"][lo:hi], dtype=np.float32)
            m[f"b{l}f"] = np.ascontiguousarray(inputs[f"b{l}"], dtype=np.float32)
        in_maps.append(m)
    return in_maps


def run(inputs, trace=False, **kw):
    from concourse.bass_utils import run_bass_kernel_spmd

    nc = _get_nc()
    in_maps = make_in_maps(inputs)
    res = run_bass_kernel_spmd(nc, in_maps, list(range(NCORES)), trace=trace, **kw)
    return res


def kernel(**inputs):
    res = run(inputs, trace=False)
    return np.asarray(res.results[0]["out"], dtype=np.float32)


# revision 16
# speedup vs baseline: 2.5320x; 2.5320x over previous
"""Trainium2 Bass kernel for a 3-layer complex RBF network.

Math per layer (complex y, G; real phi):
    dist_i = sum_j |y_j - G_ij|^2
    phi    = exp(-dist / (2 s))
    y_out  = W @ phi + b        (complex W, b)

Distribution (8 cores): shard the hidden axis I=4096 -> 512 rows of G / columns
of W per core.  dist/phi are computed fully locally per shard; the matvec
W[:, shard] @ phi_shard yields a full-length partial y that is AllReduce-summed
across cores (b is added as b/8 on every core's partial before the reduce).

Per-core implementation (NeuronCore engines):
  - G arrives natural-layout [128 i x Oprev] (2 i-chunks per DMA) via SWDGE
    cast-DMA (fp32->bf16); DVE computes diff = G - y_bcast in place; ACT
    computes Square(diff) with accum_out -> dist [128,1] chunks.
  - phi = ACT Exp(clamp(dist * (-1/(2s)))) lands as [128,1] chunks: the
    stationary (lhsT) layout the PE matvec needs.
  - W arrives as batched tiles [128 o x (4 ot x 512 i)] (SWDGE cast to bf16),
    then ONE xbar transpose per batch emits 16 transposed 128x128 blocks
    (block c2 = b*4+ic holds W^T[i-chunk ic, o-tile os*4+b]); the PE matvec
    streams rhs [128 i, (4 b x 128 o)] = N=512 per i-chunk, accumulating
    y_partial[1, 512] in PSUM.
  - AllReduce (gpsimd/ncfw) sums partial y; the y recycle for the next layer
    is a single DRAM->DRAM cast DMA (fp32->bf16) + partition-broadcast DMA.

Weights are read from HBM exactly once (fp32) -- the memory roofline.
Instruction-count discipline matters more than anything here: DMA/transpose
instructions cost ~0.6-1.2us of issuing-engine time and multi-sem waits
legalize into ~1-2us EventSemaphores, so everything is batched.

Engine rings: weight loads own the gpsimd (SWDGE) ring, ordered so the next
layer's stream sits before the current layer's AllReduce; y plumbing (ccin
store, broadcast) lives on the scalar HWDGE ring; xbar transposes own the
sync ring.
"""

import numpy as np

P = 128
NCORES = 8
HID = 4096
IS = HID // NCORES          # 512: per-core shard of the hidden axis
NCH = IS // P               # 4 chunks of 128
# (Oprev, Ol) for layers 1..3
DIMS = [(1024, 4096), (4096, 4096), (4096, 1024)]

_cache = {}


def _build_nc():
    import concourse.bacc as bacc
    import concourse.mybir as mybir
    import concourse.tile as tile

    f32 = mybir.dt.float32
    bf16 = mybir.dt.bfloat16
    AF = mybir.ActivationFunctionType
    ALU = mybir.AluOpType

    # Bacc (not raw Bass): its compile() pass legalizes multi-sem waits into
    # InstEventSemaphore carriers (HW instructions hold only 1 wait slot).
    nc = bacc.Bacc(None)

    x = nc.dram_tensor("x", [2, 1024], f32, kind="ExternalInput")
    W, G, S, B = {}, {}, {}, {}
    for l, (Op, Ol) in enumerate(DIMS, start=1):
        W[l] = nc.dram_tensor(f"W{l}s", [2, Ol, IS], f32, kind="ExternalInput")
        G[l] = nc.dram_tensor(f"G{l}s", [2, IS, Op], f32, kind="ExternalInput")
        S[l] = nc.dram_tensor(f"s{l}s", [IS], f32, kind="ExternalInput")
        B[l] = nc.dram_tensor(f"b{l}f", [2, Ol], f32, kind="ExternalInput")
    out = nc.dram_tensor("out", [2, 1024], f32, kind="ExternalOutput")

    with tile.TileContext(nc) as tc:
        with (
            tc.tile_pool(name="gnat", bufs=2) as gnat,      # [128, 2, Op] bf16
            tc.tile_pool(name="wnat", bufs=4) as wnatp,     # [128, 4, 512] bf16
            tc.tile_pool(name="wT", bufs=18) as wTp,        # [128, 16, 128] bf16
            tc.tile_pool(name="ybc", bufs=3) as ybcp,       # [128, Op] bf16
            tc.tile_pool(name="small", bufs=1) as small,
            tc.tile_pool(name="psum", bufs=4, space="PSUM") as psump,
            tc.tile_pool(name="dram", bufs=1, space="DRAM") as dramp,
        ):
            # ---------------- preloads ----------------------------------------
            # s -> -1/(2s) as [128, NCH] chunks; b -> b/8 staged in DRAM
            n2s, b8d = {}, {}
            for l, (Op, Ol) in enumerate(DIMS, start=1):
                s4 = small.tile([P, NCH], f32, tag=f"s4_{l}")
                nc.gpsimd.dma_start(s4[:], S[l][:].rearrange("(c p) -> p c", p=P))
                rec = small.tile([P, NCH], f32, tag=f"rec_{l}")
                nc.vector.reciprocal(rec[:], s4[:])
                t = small.tile([P, NCH], f32, tag=f"n2s_{l}")
                nc.vector.tensor_scalar_mul(t[:], rec[:], -0.5)
                n2s[l] = t

                bsc = small.tile([1, 2 * Ol], f32, tag="row32")  # shared scratch
                nc.scalar.dma_start(bsc[:], B[l][:])
                nc.vector.tensor_scalar_mul(bsc[:], bsc[:], 1.0 / NCORES)
                bd = dramp.tile([2, Ol], f32, tag=f"b8_{l}")
                nc.scalar.dma_start(bd[:], bsc[:])
                b8d[l] = bd

            # scratch used by "touch" ops that pre-absorb DMA-completion waits
            touch = small.tile([1, 2], bf16, tag="touch")

            # ---------------- y broadcast for layer 1 (from x, cast) ----------
            ybct = {}
            for r in range(2):
                yb = ybcp.tile([P, DIMS[0][0]], bf16, tag="ybc")
                nc.gpsimd.dma_start(yb[:], x[r : r + 1, :].partition_broadcast(P))
                nc.vector.tensor_copy(touch[:, r : r + 1], yb[0:1, 0:1])
                ybct[(1, r)] = yb

            # ---------------- weight-load emission helpers --------------------
            gt = {}    # (l, r, cp) -> [128, 2, Op] natural G tile (chunks 2cp, 2cp+1)
            wTt = {}   # (l, r, os) -> [128, 16, 128] transposed W batch

            def emit_g_loads(l):
                Op = DIMS[l - 1][0]
                for cp in range(NCH // 2):
                    for r in range(2):
                        g = gnat.tile([P, 2, Op], bf16, tag="gnat")
                        nc.gpsimd.dma_start(
                            g[:],
                            G[l][r, cp * 2 * P : (cp + 1) * 2 * P, :].rearrange(
                                "(c p) j -> p c j", p=P
                            ),
                        )
                        gt[(l, r, cp)] = g

            def emit_w_loads(l, ros):
                """Batched SWDGE cast-load + ONE xbar transpose per o-slice."""
                for r, os_ in ros:
                    wn = wnatp.tile([P, 4, 512], bf16, tag="wnat")
                    nc.gpsimd.dma_start(
                        wn[:],
                        W[l][r, os_ * 512 : (os_ + 1) * 512, :].rearrange(
                            "(b p) i -> p b i", p=P
                        ),
                    )
                    wt = wTp.tile([P, 16, P], bf16, tag="wT")
                    nc.sync.dma_start(wt[:], wn[:], transpose=True)
                    wTt[(l, r, os_)] = wt

            def w_ros(l, lo, hi):
                Ol = DIMS[l - 1][1]
                allr = [(r, os_) for r in range(2) for os_ in range(Ol // 512)]
                return allr[lo:hi]

            # Prefetch order on the gpsimd ring:
            # G1, W1, G2, W2[:6] | b1acc AR1 | W2[6:], ybf1, G3, W3[:2] | b2acc
            # AR2 | W3[2:], ybf2 | b3acc AR3 | out
            emit_g_loads(1)
            emit_w_loads(1, w_ros(1, 0, 10**9))
            emit_g_loads(2)
            emit_w_loads(2, w_ros(2, 0, 6))

            # ---------------- per-layer compute --------------------------------
            ysrc = None  # DRAM bf16 [2, Op] holding y of the previous layer
            for l, (Op, Ol) in enumerate(DIMS, start=1):
                NOS = Ol // 512

                if l > 1:
                    for r in range(2):
                        yb = ybcp.tile([P, Op], bf16, tag="ybc")
                        nc.scalar.dma_start(
                            yb[:], ysrc[r : r + 1, :].partition_broadcast(P)
                        )
                        nc.vector.tensor_copy(touch[:, r : r + 1], yb[0:1, 0:1])
                        ybct[(l, r)] = yb

                # ---- dist: g = G - y_bcast (DVE, in place), Square+accum (ACT)
                dacc = small.tile([P, 2 * NCH], f32, tag=f"dacc_{l}")
                for cp in range(NCH // 2):
                    for r in range(2):
                        g = gt[(l, r, cp)]
                        for ci in range(2):
                            c = 2 * cp + ci
                            gs = g[:, ci, :]
                            nc.vector.tensor_sub(gs, gs, ybct[(l, r)][:])
                            nc.scalar.activation(
                                gs, gs, AF.Square,
                                accum_out=dacc[:, 2 * c + r : 2 * c + r + 1],
                            )

                # ---- phi = exp(clamp((d_re+d_im) * -1/(2s), -85)) ----
                phi = small.tile([P, NCH], bf16, tag=f"phi_{l}")
                expin = small.tile([P, NCH], f32, tag=f"expin_{l}")
                junk2 = small.tile([P, 2], f32, tag=f"junk_{l}")
                for c in range(NCH):
                    nc.vector.tensor_scalar(
                        junk2[:], dacc[:, 2 * c : 2 * c + 2],
                        n2s[l][:, c : c + 1], 0.0, ALU.mult, ALU.add,
                        accum_out=expin[:, c : c + 1],
                    )
                    nc.vector.tensor_scalar_max(
                        expin[:, c : c + 1], expin[:, c : c + 1], -85.0
                    )
                    nc.scalar.activation(
                        phi[:, c : c + 1], expin[:, c : c + 1], AF.Exp
                    )

                # ---- y_partial = W_shard @ phi (PE), PSUM -> SBUF (ACT) ----
                # single-row layout [1, 2*Ol]: row = comp0 ++ comp1
                ysb = small.tile([1, 2 * Ol], f32, tag="row32")
                for r in range(2):
                    for os_ in range(NOS):
                        wt = wTt[(l, r, os_)]
                        # rhs for i-chunk ic: blocks c2 = b*4+ic, b=0..3
                        w4 = wt[:].rearrange("p (b ic) f -> p ic b f", ic=NCH)
                        ps = psump.tile([1, 512], f32, tag="psy")
                        for ic in range(NCH):
                            nc.tensor.matmul(
                                ps[:],
                                phi[:, ic : ic + 1],
                                w4[:, ic, :, :],
                                start=(ic == 0),
                                stop=(ic == NCH - 1),
                            )
                        off = r * Ol + os_ * 512
                        # scalar engine: merges the matmuls' phi-wait and the
                        # psum-slot WAR wait onto one engine clock
                        nc.scalar.copy(ysb[:, off : off + 512], ps[:])

                # ---- partial y + b/8, then AllReduce across the 8 cores ----
                ccin = dramp.tile([2, Ol], f32, tag=f"ccin_{l}")
                ccout = dramp.tile([2, Ol], f32, tag=f"ccout_{l}")
                nc.scalar.dma_start(ccin[:], ysb[:])
                nc.gpsimd.dma_start(ccin[:], b8d[l][:], accum_op=ALU.add)
                nc.gpsimd.collective_compute(
                    "AllReduce",
                    ALU.add,
                    replica_groups=[list(range(NCORES))],
                    ins=[ccin.opt()],
                    outs=[ccout.opt()],
                )
                # keep next-layer weight prefetch flowing on the gpsimd ring;
                # the DRAM->DRAM y cast must come before anything that
                # transitively feeds the layer after next (deadlock audit in
                # the module docstring notes)
                if l == 1:
                    emit_w_loads(2, w_ros(2, 6, 10**9))
                    ybf = dramp.tile([2, Ol], bf16, tag=f"ybf_{l}")
                    nc.gpsimd.dma_start(ybf[:], ccout[:])  # cast f32->bf16
                    ysrc = ybf
                    emit_g_loads(3)
                    emit_w_loads(3, w_ros(3, 0, 2))
                elif l == 2:
                    emit_w_loads(3, w_ros(3, 2, 10**9))
                    ybf = dramp.tile([2, Ol], bf16, tag=f"ybf_{l}")
                    nc.gpsimd.dma_start(ybf[:], ccout[:])  # cast f32->bf16
                    ysrc = ybf
                else:
                    nc.gpsimd.dma_start(out[:], ccout[:])

    # Bacc.finalize runs compile(): reg alloc + event-semaphore legalization
    nc.finalize()
    return nc


def _get_nc():
    if "nc" not in _cache:
        _cache["nc"] = _build_nc()
    return _cache["nc"]


def make_in_maps(inputs):
    """Host-side sharding: slice the hidden axis into 8 shards."""
    in_maps = []
    for c in range(NCORES):
        lo, hi = c * IS, (c + 1) * IS
        m = {"x": np.ascontiguousarray(inputs["x"], dtype=np.float32)}
        for l in range(1, 4):
            m[f"W{l}s"] = np.ascontiguousarray(inputs[f"W{l}"][:, :, lo:hi], dtype=np.float32)
            m[f"G{l}s"] = np.ascontiguousarray(inputs[f"G{l}"][:, lo:hi, :], dtype=np.float32)
            m[f"s{l}s"] = np.ascontiguousarray(inputs[f"s{l}"][lo:hi], dtype=np.float32)
            m[f"b{l}f"] = np.ascontiguousarray(inputs[f"b{l}"], dtype=np.float32)
        in_maps.append(m)
    return in_maps


def run(inputs, trace=False, **kw):
    from concourse.bass_utils import run_bass_kernel_spmd

    nc = _get_nc()
    in_maps = make_in_maps(inputs)
    res = run_bass_kernel_spmd(nc, in_maps, list(range(NCORES)), trace=trace, **kw)
    return res


def kernel(**inputs):
    res = run(inputs, trace=False)
    return np.asarray(res.results[0]["out"], dtype=np.float32)
